# revision 8
# baseline (speedup 1.0000x reference)
"""AdaptiveMultiSiren Trainium2 kernel.

Per-block SIREN MLP (3 -> 64 -> 64 -> 64 -> 3, sin(30*x) activations) applied
to 2048 routed blocks of 1024 coords each. Data-parallel over blocks across
8 NeuronCores (256 blocks / core); the host-side gather of per-block weights
IS the shard construction. Two blocks pack per matmul block-diagonally so
TensorE/ScalarE run at the full 128-partition width.

Per pair of blocks (a, b), activations live as [features, T] in SBUF:
  matmul(out, lhsT, rhs) = lhsT.T @ rhs, K = contraction on partitions.
  Weights are pre-scaled by 30/2pi so psum t = z/2pi. The sin LUT only
  covers [-pi, pi], so each sin layer does explicit range reduction:
    main matmul        t = W~.T h           (psum, fp32)
    DVE round          n = (t + M) - M      (M = 1.5*2^23 magic, n = round(t))
    PE accumulate      r = t - n            (lhsT = -Identity, start=False)
    ScalarE            h' = Sin(2pi*r + 30*b)   (per-partition bias AP)
  L0 folds its bias into the matmul via K-augmentation (ones rows in x).
  L3 (no sin) col-tiles 4 pairs into one PSUM tile at partition offsets
  0/32/64/96, evacuated by one VectorE tensor_scalar_add (+b3) per 4 pairs.
"""

import sys

if "/opt/trn_rl_repo" not in sys.path:
    sys.path.insert(0, "/opt/trn_rl_repo")

import numpy as np

C, B, T = 4096, 2048, 1024
DIN, DH, DOUT = 3, 64, 3
OMEGA0 = 30.0
TWO_PI = float(2 * np.pi)
SCALE = OMEGA0 / TWO_PI
MAGIC = float(1.5 * 2 ** 23)
N_CORES = 8
BPC = B // N_CORES          # blocks per core (256)
NPAIR = BPC // 2            # block pairs per core (128)
G = 4                       # pairs per group (weight DMA + shared L3 psum)
NG = NPAIR // G             # 32

_CACHE = {}


def _build():
    """Build + compile the per-core NEFF (same SPMD program on all cores)."""
    import concourse.tile as tile
    from concourse import bacc, mybir

    f32 = mybir.dt.float32
    f32r = mybir.dt.float32r
    Sin = mybir.ActivationFunctionType.Sin
    Identity = mybir.ActivationFunctionType.Identity
    Alu = mybir.AluOpType

    nc = bacc.Bacc("TRN2", target_bir_lowering=False, debug=False,
                   num_devices=N_CORES)

    xT = nc.dram_tensor("xT", [NPAIR * 8, T], f32r, kind="ExternalInput").ap()
    w0 = nc.dram_tensor("w0", [8, NPAIR * 128], f32r, kind="ExternalInput").ap()
    w1 = nc.dram_tensor("w1", [128, NPAIR * 128], f32r, kind="ExternalInput").ap()
    w2 = nc.dram_tensor("w2", [128, NPAIR * 128], f32r, kind="ExternalInput").ap()
    w3 = nc.dram_tensor("w3", [128, NPAIR * 6], f32r, kind="ExternalInput").ap()
    negI = nc.dram_tensor("negI", [128, 128], f32r, kind="ExternalInput").ap()
    bias = nc.dram_tensor("bias", [128, NPAIR * 2], f32, kind="ExternalInput").ap()
    b3 = nc.dram_tensor("b3", [6, NPAIR], f32, kind="ExternalInput").ap()
    out = nc.dram_tensor("out", [NPAIR * 6, T], f32, kind="ExternalOutput").ap()

    with tile.TileContext(nc) as tc:
        with (
            tc.tile_pool(name="const", bufs=1) as constp,
            tc.tile_pool(name="wg", bufs=3) as wgp,
            tc.tile_pool(name="xp", bufs=6) as xp,
            tc.tile_pool(name="hp", bufs=6) as hp,
            tc.tile_pool(name="np_", bufs=3) as npp,
            tc.tile_pool(name="yp", bufs=3) as yp,
            tc.tile_pool(name="ps", bufs=4, space="PSUM") as psp,
        ):
            w3_t = constp.tile([128, NPAIR * 6], f32r)
            nc.sync.dma_start(out=w3_t[:], in_=w3[:])
            bias_t = constp.tile([128, NPAIR * 2], f32)
            nc.sync.dma_start(out=bias_t[:], in_=bias[:])
            b3_t = constp.tile([6, NPAIR], f32)
            nc.sync.dma_start(out=b3_t[:], in_=b3[:])
            nI_t = constp.tile([128, 128], f32r)
            nc.sync.dma_start(out=nI_t[:], in_=negI[:])
            zero_t = constp.tile([128, 1], f32)
            nc.vector.memset(zero_t[:], 0.0)

            for g in range(NG):
                gs = g * G * 128
                w0_t = wgp.tile([8, G * 128], f32r, tag="w0g")
                nc.sync.dma_start(out=w0_t[:], in_=w0[:, gs:gs + G * 128])
                w1_t = wgp.tile([128, G * 128], f32r, tag="w1g")
                nc.sync.dma_start(out=w1_t[:], in_=w1[:, gs:gs + G * 128])
                w2_t = wgp.tile([128, G * 128], f32r, tag="w2g")
                nc.sync.dma_start(out=w2_t[:], in_=w2[:, gs:gs + G * 128])

                for j in range(G):
                    p = g * G + j
                    js = j * 128

                    x_t = xp.tile([8, T], f32r)
                    nc.sync.dma_start(out=x_t[:], in_=xT[p * 8:(p + 1) * 8, :])

                    h_prev = x_t
                    for li, (w_t, kdim) in enumerate(
                            ((w0_t, 8), (w1_t, 128), (w2_t, 128))):
                        ps = psp.tile([128, T], f32, tag="ps")
                        for c in range(2):
                            cs = c * 512
                            nc.tensor.matmul(
                                ps[:, cs:cs + 512],
                                w_t[0:kdim, js:js + 128],
                                h_prev[0:kdim, cs:cs + 512],
                                start=True, stop=False)
                        n_t = npp.tile([128, T], f32r, tag="n")
                        nc.vector.tensor_scalar(n_t[:], ps[:], MAGIC, MAGIC,
                                                Alu.add, Alu.subtract)
                        for c in range(2):
                            cs = c * 512
                            nc.tensor.matmul(
                                ps[:, cs:cs + 512],
                                nI_t[:],
                                n_t[:, cs:cs + 512],
                                start=False, stop=True)
                        h_t = hp.tile([128, T], f32r, tag="h")
                        if li == 0:
                            b_ap = zero_t[:, 0:1]
                        else:
                            b_ap = bias_t[:, p * 2 + li - 1:p * 2 + li]
                        nc.scalar.activation(h_t[:], ps[:], Sin,
                                             bias=b_ap, scale=TWO_PI)
                        h_prev = h_t

                    ps3 = psp.tile([128, T], f32, tag="ps")
                    for c in range(2):
                        cs = c * 512
                        nc.tensor.matmul(
                            ps3[0:6, cs:cs + 512],
                            w3_t[:, p * 6:(p + 1) * 6],
                            h_prev[:, cs:cs + 512],
                            start=True, stop=True)
                    y_t = yp.tile([6, T], f32)
                    if p % 2 == 0:
                        nc.vector.tensor_scalar(y_t[:], ps3[0:6, :],
                                                b3_t[:, p:p + 1], None, Alu.add)
                    else:
                        nc.scalar.activation(y_t[:], ps3[0:6, :], Identity,
                                             bias=b3_t[:, p:p + 1], scale=1.0)
                    nc.sync.dma_start(out=out[p * 6:(p + 1) * 6, :],
                                      in_=y_t[:])

    nc.compile()
    return nc


def _get_nc():
    if "nc" not in _CACHE:
        _CACHE["nc"] = _build()
    return _CACHE["nc"]


def _prep_core(ids, inp, W0, b0, W1, b1, W2, b2, W3, b3):
    """Build one core's input map: gather + pair-pack the active blocks."""
    f = np.float32
    ev, od = ids[0::2], ids[1::2]

    # x-augmented: per pair rows [xa(3); xb(3); 1; 1]
    xg = inp[ids].transpose(0, 2, 1)                  # [BPC, 3, T]
    xTp = np.empty((NPAIR, 8, T), f)
    xTp[:, 0:3] = xg[0::2]
    xTp[:, 3:6] = xg[1::2]
    xTp[:, 6:8] = 1.0
    xT = np.ascontiguousarray(xTp).reshape(NPAIR * 8, T)

    # W0 augmented with bias rows, scaled by 30/2pi
    w0p = np.zeros((NPAIR, 8, 128), f)
    w0p[:, 0:3, 0:64] = SCALE * W0[ev]
    w0p[:, 3:6, 64:128] = SCALE * W0[od]
    w0p[:, 6, 0:64] = SCALE * b0.reshape(-1, DH)[ev]
    w0p[:, 7, 64:128] = SCALE * b0.reshape(-1, DH)[od]
    w0l = np.ascontiguousarray(w0p.transpose(1, 0, 2)).reshape(8, NPAIR * 128)

    def diag128(Wt):
        wp = np.zeros((NPAIR, 128, 128), f)
        wp[:, 0:64, 0:64] = SCALE * Wt[ev]
        wp[:, 64:128, 64:128] = SCALE * Wt[od]
        return np.ascontiguousarray(wp.transpose(1, 0, 2)).reshape(128, NPAIR * 128)

    w1l = diag128(W1)
    w2l = diag128(W2)

    w3p = np.zeros((NPAIR, 128, 6), f)
    w3p[:, 0:64, 0:3] = W3[ev]
    w3p[:, 64:128, 3:6] = W3[od]
    w3l = np.ascontiguousarray(w3p.transpose(1, 0, 2)).reshape(128, NPAIR * 6)

    negI = np.ascontiguousarray(-np.eye(128, dtype=f))

    # ACT biases for sin layers 1,2: 30*b, pair-stacked on partitions
    biasp = np.empty((NPAIR, 2, 128), f)
    for l, bl in enumerate((b1, b2)):
        bl2 = bl.reshape(-1, DH)
        biasp[:, l, 0:64] = OMEGA0 * bl2[ev]
        biasp[:, l, 64:128] = OMEGA0 * bl2[od]
    biasl = np.ascontiguousarray(biasp.transpose(2, 0, 1)).reshape(128, NPAIR * 2)

    b3v = b3.reshape(-1, DOUT)
    b3p = np.empty((NPAIR, 6), f)
    b3p[:, 0:3] = b3v[ev]
    b3p[:, 3:6] = b3v[od]
    b3l = np.ascontiguousarray(b3p.T)                 # [6, NPAIR]

    return {"xT": xT, "w0": w0l, "w1": w1l, "w2": w2l, "w3": w3l,
            "negI": negI, "bias": biasl, "b3": b3l}


def make_in_maps(inp, indices, W0, b0, W1, b1, W2, b2, W3, b3):
    inp = np.asarray(inp, dtype=np.float32)
    idx = np.asarray(indices).astype(np.int64)
    args = tuple(np.asarray(a, dtype=np.float32)
                 for a in (W0, b0, W1, b1, W2, b2, W3, b3))
    return [
        _prep_core(idx[i * BPC:(i + 1) * BPC], inp, *args)
        for i in range(N_CORES)
    ]


def unshard(results):
    shards = []
    for i in range(N_CORES):
        y = results[i]["out"].reshape(BPC, DOUT, T)
        shards.append(y.transpose(0, 2, 1))           # [BPC, T, 3]
    return np.ascontiguousarray(np.concatenate(shards, axis=0))


def kernel(inp, indices, W0, b0, W1, b1, W2, b2, W3, b3):
    from concourse.bass_utils import run_bass_kernel_spmd

    nc = _get_nc()
    in_maps = make_in_maps(inp, indices, W0, b0, W1, b1, W2, b2, W3, b3)
    res = run_bass_kernel_spmd(nc, in_maps, core_ids=list(range(N_CORES)))
    return unshard(res.results)


# revision 10
# speedup vs baseline: 2.7500x; 2.7500x over previous
"""AdaptiveMultiSiren Trainium2 kernel.

Per-block SIREN MLP (3 -> 64 -> 64 -> 64 -> 3, sin(30*x) activations) applied
to 2048 routed blocks of 1024 coords each. Data-parallel over blocks across
8 NeuronCores (256 blocks / core); the host-side gather of per-block weights
IS the shard construction. Two blocks pack per matmul block-diagonally so
TensorE/ScalarE run at the full 128-partition width.

Per pair of blocks (a, b), activations live as [features, T] in SBUF:
  matmul(out, lhsT, rhs) = lhsT.T @ rhs, K = contraction on partitions.
  Weights are pre-scaled by 30/2pi so psum t = z/2pi. The sin LUT only
  covers [-pi, pi], so each sin layer does explicit range reduction:
    main matmul        t = W~.T h           (psum, fp32)
    DVE round          n = (t + M) - M      (M = 1.5*2^23 magic, n = round(t))
    PE accumulate      r = t - n            (lhsT = -Identity, start=False)
    ScalarE            h' = Sin(2pi*r + 30*b)   (per-partition bias AP)
  L0 folds its bias into the matmul via K-augmentation (ones rows in x).
  L3 (no sin) col-tiles 4 pairs into one PSUM tile at partition offsets
  0/32/64/96, evacuated by one VectorE tensor_scalar_add (+b3) per 4 pairs.
"""

import sys

if "/opt/trn_rl_repo" not in sys.path:
    sys.path.insert(0, "/opt/trn_rl_repo")

import numpy as np

C, B, T = 4096, 2048, 1024
DIN, DH, DOUT = 3, 64, 3
OMEGA0 = 30.0
TWO_PI = float(2 * np.pi)
SCALE = OMEGA0 / TWO_PI
MAGIC = float(1.5 * 2 ** 23)
N_CORES = 8
BPC = B // N_CORES          # blocks per core (256)
NPAIR = BPC // 2            # block pairs per core (128)
G = 4                       # pairs per group (weight DMA + shared L3 psum)
NG = NPAIR // G             # 32

_CACHE = {}


def _build():
    """Build + compile the per-core NEFF (same SPMD program on all cores)."""
    import concourse.tile as tile
    from concourse import bacc, mybir

    f32 = mybir.dt.float32
    f32r = mybir.dt.float32r
    Sin = mybir.ActivationFunctionType.Sin
    Identity = mybir.ActivationFunctionType.Identity
    Alu = mybir.AluOpType

    nc = bacc.Bacc("TRN2", target_bir_lowering=False, debug=False,
                   num_devices=N_CORES)

    xT = nc.dram_tensor("xT", [NPAIR * 8, T], f32r, kind="ExternalInput").ap()
    w0 = nc.dram_tensor("w0", [8, NPAIR * 128], f32r, kind="ExternalInput").ap()
    w1 = nc.dram_tensor("w1", [128, NPAIR * 128], f32r, kind="ExternalInput").ap()
    w2 = nc.dram_tensor("w2", [128, NPAIR * 128], f32r, kind="ExternalInput").ap()
    w3 = nc.dram_tensor("w3", [128, NPAIR * 6], f32r, kind="ExternalInput").ap()
    negI = nc.dram_tensor("negI", [128, 128], f32r, kind="ExternalInput").ap()
    bias = nc.dram_tensor("bias", [128, NPAIR * 2], f32, kind="ExternalInput").ap()
    b3 = nc.dram_tensor("b3", [6, NPAIR], f32, kind="ExternalInput").ap()
    out = nc.dram_tensor("out", [NPAIR * 6, T], f32, kind="ExternalOutput").ap()

    with tile.TileContext(nc) as tc:
        with (
            tc.tile_pool(name="const", bufs=1) as constp,
            tc.tile_pool(name="wg", bufs=3) as wgp,
            tc.tile_pool(name="xp", bufs=2) as xp,
            tc.tile_pool(name="hp", bufs=2) as hp,
            tc.tile_pool(name="np_", bufs=2) as npp,
            tc.tile_pool(name="yp", bufs=2) as yp,
            tc.tile_pool(name="ps", bufs=1, space="PSUM") as psp,
        ):
            w3_t = constp.tile([128, NPAIR * 6], f32r)
            nc.sync.dma_start(out=w3_t[:], in_=w3[:])
            bias_t = constp.tile([128, NPAIR * 2], f32)
            nc.sync.dma_start(out=bias_t[:], in_=bias[:])
            b3_t = constp.tile([6, NPAIR], f32)
            nc.sync.dma_start(out=b3_t[:], in_=b3[:])
            nI_t = constp.tile([128, 128], f32r)
            nc.sync.dma_start(out=nI_t[:], in_=negI[:])
            zero_t = constp.tile([128, 1], f32)
            nc.vector.memset(zero_t[:], 0.0)

            # wave = G pairs processed breadth-first per layer so every
            # engine's instruction stream stays dense (in-order engines).
            # Two pairs share one [128, 2048] PSUM tile (4 banks) so the
            # DVE round runs once per 2 pairs.
            for g in range(NG):
                gs = g * G * 128
                w0_t = wgp.tile([8, G * 128], f32r, tag="w0g")
                nc.sync.dma_start(out=w0_t[:], in_=w0[:, gs:gs + G * 128])
                w1_t = wgp.tile([128, G * 128], f32r, tag="w1g")
                nc.sync.dma_start(out=w1_t[:], in_=w1[:, gs:gs + G * 128])
                w2_t = wgp.tile([128, G * 128], f32r, tag="w2g")
                nc.sync.dma_start(out=w2_t[:], in_=w2[:, gs:gs + G * 128])

                x_ts = []
                for j in range(G):
                    p = g * G + j
                    x_t = xp.tile([8, T], f32r, tag=f"x{j}")
                    nc.sync.dma_start(out=x_t[:], in_=xT[p * 8:(p + 1) * 8, :])
                    x_ts.append(x_t)

                h_prev = x_ts
                for li, (w_t, kdim) in enumerate(
                        ((w0_t, 8), (w1_t, 128), (w2_t, 128))):
                    ps_ts = []
                    for half in range(G // 2):
                        ps = psp.tile([128, 2 * T], f32, tag=f"ps{half}")
                        ps_ts.append(ps)
                        for jj in range(2):
                            j = half * 2 + jj
                            nc.tensor.matmul(
                                ps[:, jj * T:jj * T + 512],
                                w_t[0:kdim, j * 128:j * 128 + 128],
                                h_prev[j][0:kdim, 0:512],
                                start=True, stop=False)
                            nc.tensor.matmul(
                                ps[:, jj * T + 512:jj * T + 1024],
                                w_t[0:kdim, j * 128:j * 128 + 128],
                                h_prev[j][0:kdim, 512:1024],
                                start=True, stop=False)
                    n_ts = []
                    for half in range(G // 2):
                        n_t = npp.tile([128, 2 * T], f32r, tag=f"n{half}")
                        nc.vector.tensor_scalar(n_t[:], ps_ts[half][:],
                                                MAGIC, MAGIC,
                                                Alu.add, Alu.subtract)
                        n_ts.append(n_t)
                    for half in range(G // 2):
                        for cs in (0, 512, 1024, 1536):
                            nc.tensor.matmul(
                                ps_ts[half][:, cs:cs + 512],
                                nI_t[:],
                                n_ts[half][:, cs:cs + 512],
                                start=False, stop=True)
                    h_ts = []
                    for j in range(G):
                        p = g * G + j
                        half, jj = j // 2, j % 2
                        h_t = hp.tile([128, T], f32r, tag=f"h{j}")
                        if li == 0:
                            b_ap = zero_t[:, 0:1]
                        else:
                            b_ap = bias_t[:, p * 2 + li - 1:p * 2 + li]
                        nc.scalar.activation(
                            h_t[:], ps_ts[half][:, jj * T:(jj + 1) * T],
                            Sin, bias=b_ap, scale=TWO_PI)
                        h_ts.append(h_t)
                    h_prev = h_ts

                ps3_ts = []
                for half in range(G // 2):
                    ps3 = psp.tile([128, 2 * T], f32, tag=f"ps{half}")
                    ps3_ts.append(ps3)
                    for jj in range(2):
                        j = half * 2 + jj
                        p = g * G + j
                        for c in range(2):
                            cs = jj * T + c * 512
                            nc.tensor.matmul(
                                ps3[0:6, cs:cs + 512],
                                w3_t[:, p * 6:(p + 1) * 6],
                                h_prev[j][:, c * 512:c * 512 + 512],
                                start=True, stop=True)
                for j in range(G):
                    p = g * G + j
                    half, jj = j // 2, j % 2
                    src = ps3_ts[half][0:6, jj * T:(jj + 1) * T]
                    y_t = yp.tile([6, T], f32, tag=f"y{j}")
                    if j % 2 == 0:
                        nc.vector.tensor_scalar(y_t[:], src,
                                                b3_t[:, p:p + 1], None, Alu.add)
                    else:
                        nc.scalar.activation(y_t[:], src, Identity,
                                             bias=b3_t[:, p:p + 1], scale=1.0)
                    nc.sync.dma_start(out=out[p * 6:(p + 1) * 6, :],
                                      in_=y_t[:])

    nc.compile()
    return nc


def _get_nc():
    if "nc" not in _CACHE:
        _CACHE["nc"] = _build()
    return _CACHE["nc"]


def _prep_core(ids, inp, W0, b0, W1, b1, W2, b2, W3, b3):
    """Build one core's input map: gather + pair-pack the active blocks."""
    f = np.float32
    ev, od = ids[0::2], ids[1::2]

    # x-augmented: per pair rows [xa(3); xb(3); 1; 1]
    xg = inp[ids].transpose(0, 2, 1)                  # [BPC, 3, T]
    xTp = np.empty((NPAIR, 8, T), f)
    xTp[:, 0:3] = xg[0::2]
    xTp[:, 3:6] = xg[1::2]
    xTp[:, 6:8] = 1.0
    xT = np.ascontiguousarray(xTp).reshape(NPAIR * 8, T)

    # W0 augmented with bias rows, scaled by 30/2pi
    w0p = np.zeros((NPAIR, 8, 128), f)
    w0p[:, 0:3, 0:64] = SCALE * W0[ev]
    w0p[:, 3:6, 64:128] = SCALE * W0[od]
    w0p[:, 6, 0:64] = SCALE * b0.reshape(-1, DH)[ev]
    w0p[:, 7, 64:128] = SCALE * b0.reshape(-1, DH)[od]
    w0l = np.ascontiguousarray(w0p.transpose(1, 0, 2)).reshape(8, NPAIR * 128)

    def diag128(Wt):
        wp = np.zeros((NPAIR, 128, 128), f)
        wp[:, 0:64, 0:64] = SCALE * Wt[ev]
        wp[:, 64:128, 64:128] = SCALE * Wt[od]
        return np.ascontiguousarray(wp.transpose(1, 0, 2)).reshape(128, NPAIR * 128)

    w1l = diag128(W1)
    w2l = diag128(W2)

    w3p = np.zeros((NPAIR, 128, 6), f)
    w3p[:, 0:64, 0:3] = W3[ev]
    w3p[:, 64:128, 3:6] = W3[od]
    w3l = np.ascontiguousarray(w3p.transpose(1, 0, 2)).reshape(128, NPAIR * 6)

    negI = np.ascontiguousarray(-np.eye(128, dtype=f))

    # ACT biases for sin layers 1,2: 30*b, pair-stacked on partitions
    biasp = np.empty((NPAIR, 2, 128), f)
    for l, bl in enumerate((b1, b2)):
        bl2 = bl.reshape(-1, DH)
        biasp[:, l, 0:64] = OMEGA0 * bl2[ev]
        biasp[:, l, 64:128] = OMEGA0 * bl2[od]
    biasl = np.ascontiguousarray(biasp.transpose(2, 0, 1)).reshape(128, NPAIR * 2)

    b3v = b3.reshape(-1, DOUT)
    b3p = np.empty((NPAIR, 6), f)
    b3p[:, 0:3] = b3v[ev]
    b3p[:, 3:6] = b3v[od]
    b3l = np.ascontiguousarray(b3p.T)                 # [6, NPAIR]

    return {"xT": xT, "w0": w0l, "w1": w1l, "w2": w2l, "w3": w3l,
            "negI": negI, "bias": biasl, "b3": b3l}


def make_in_maps(inp, indices, W0, b0, W1, b1, W2, b2, W3, b3):
    inp = np.asarray(inp, dtype=np.float32)
    idx = np.asarray(indices).astype(np.int64)
    args = tuple(np.asarray(a, dtype=np.float32)
                 for a in (W0, b0, W1, b1, W2, b2, W3, b3))
    return [
        _prep_core(idx[i * BPC:(i + 1) * BPC], inp, *args)
        for i in range(N_CORES)
    ]


def unshard(results):
    shards = []
    for i in range(N_CORES):
        y = results[i]["out"].reshape(BPC, DOUT, T)
        shards.append(y.transpose(0, 2, 1))           # [BPC, T, 3]
    return np.ascontiguousarray(np.concatenate(shards, axis=0))


def kernel(inp, indices, W0, b0, W1, b1, W2, b2, W3, b3):
    from concourse.bass_utils import run_bass_kernel_spmd

    nc = _get_nc()
    in_maps = make_in_maps(inp, indices, W0, b0, W1, b1, W2, b2, W3, b3)
    res = run_bass_kernel_spmd(nc, in_maps, core_ids=list(range(N_CORES)))
    return unshard(res.results)
